# revision 7
# baseline (speedup 1.0000x reference)
"""Column-parallel linear Y = X @ W^T + b on 8 Trainium2 NeuronCores.

Strategy: sequence-shard X across the 8 cores (4096 tokens each); every core
holds the full weight, computes its token slab against all 4096 output
features, so no collective is needed and no core re-reads another's tokens.

v3 (bf16 + fast start): operands are bf16 (norm rel err ~2.6e-3, well
inside the 2e-2 gate); output returns bf16 and the host upcasts.

DMA layouts give every transfer 8KB-contiguous runs per partition (the HW
DMA costs ~19ns per descriptor, so 1KB-element transfers crawl at
~55GB/s while 8KB-element ones hit wire rate):
  xT [8, 128, 8, 512]  bf16  xT[g, p, ko, m'] = X_shard[g*512+m', ko*128+p]
  wT [8, 128, 8, 512]  bf16  wT[nc, p, ko, q] = W[nc*512+q, ko*128+p]
  bias [4096]          fp32
  out [128, 32, 4096]  bf16  out[p, mi, n] = Y_shard[mi*128+p, n]

Schedule:
  - ~32 dummy N=128 matmuls on a zeroed scratch tile run during the input
    DMA wait so the PE_HAM clock gate is already released (2.4 GHz) when
    the first real matmul issues.
  - Phase A (m-tiles 0..3) iterates n-chunk-major, consuming each 1MB W
    chunk as it lands: compute starts after ~2MB of input instead of the
    full 17MB.
  - Phase B (m-tiles 4..31) is the steady state: stationary = X m-tile
    [128k,128m], moving = W [128k,512n]; each half of the n-range
    accumulates over the 8 k-tiles into 4 PSUM banks while the other
    half's PSUM is evicted (DVE bias-add + bf16 cast) - ping-pong keeps
    the PE streaming back-to-back at 216ns/matmul.
  - Output DMAs fire at half-row granularity on alternating rings.
"""

import numpy as np
import ml_dtypes

import concourse.bass as bass
import concourse.mybir as mybir
import concourse.tile as tile
from concourse import bacc
from concourse.bass_utils import run_bass_kernel_spmd

P = 128
SEQ, BATCH, D_IN, D_OUT = 8192, 4, 1024, 4096
N_CORES = 8
TOK = SEQ * BATCH
TOK_SHARD = TOK // N_CORES     # 4096
KO = D_IN // P                 # 8
M_TILES = TOK_SHARD // P       # 32 m-tiles of 128 tokens
NCHUNK = 512                   # moving-operand width (walrus ISA cap)
N_CHUNKS = D_OUT // NCHUNK     # 8
XG = 4                         # m-tiles per X DMA group
G = M_TILES // XG              # 8 X groups
GW = XG * P                    # 512 tokens per group
PHASE_A_MI = 4                 # m-tiles computed n-chunk-major during W load
N_WARM = 32                    # dummy matmuls to pre-release the HAM clock gate

_CACHE = {}

# Last BassKernelResults, for test harnesses that want exec_time_ns.
LAST_RESULT = None


def _build():
    if "nc" in _CACHE:
        return _CACHE["nc"], _CACHE["names"]

    nc = bacc.Bacc(None, target_bir_lowering=False, debug=False)
    with tile.TileContext(nc) as tc:
        with (
            tc.tile_pool(name="dram", bufs=1, space="DRAM") as dram,
            tc.tile_pool(name="consts", bufs=1) as consts,
            tc.tile_pool(name="opool", bufs=6) as opool,
            tc.tile_pool(name="pspool", bufs=8, space="PSUM") as pspool,
        ):
            xT = dram.tile((G, P, KO, GW), mybir.dt.bfloat16, kind="ExternalInput")
            wT = dram.tile(
                (N_CHUNKS, P, KO, NCHUNK), mybir.dt.bfloat16, kind="ExternalInput"
            )
            bias_d = dram.tile((D_OUT,), mybir.dt.float32, kind="ExternalInput")
            out = dram.tile(
                (P, M_TILES, D_OUT), mybir.dt.bfloat16, kind="ExternalOutput"
            )

            # Warm-up scratch: zeroed once by DVE, then N_WARM tiny matmuls
            # keep the PE busy while the first input DMAs land.
            scratch = consts.tile([P, P], mybir.dt.bfloat16, name="scratch")
            nc.vector.memset(scratch[:], 0)
            ps_warm = pspool.tile([P, NCHUNK], mybir.dt.float32, name="ps")
            for _ in range(N_WARM):
                nc.tensor.matmul(
                    ps_warm[:, 0:P], scratch[:], scratch[:], start=True, stop=True
                )

            # bias broadcast to every partition so the evict add is a plain
            # elementwise tensor_tensor
            bias_sb = consts.tile([P, D_OUT], mybir.dt.float32, name="bias_sb")
            bias_bcast = bass.AP(
                tensor=bias_d.tensor,
                offset=bias_d.offset,
                ap=[[0, P], *bias_d.ap],
            )
            nc.gpsimd.dma_start(out=bias_sb[:], in_=bias_bcast)

            # Input DMAs: emission order == per-ring arrival order. The first
            # X group and W chunk lead their rings; W chunks arrive ~every
            # 3us, phase A consumes one every ~6.8us.
            wc = [None] * N_CHUNKS
            xt = [None] * G

            def load_w(ncix, eng):
                t = consts.tile([P, KO, NCHUNK], mybir.dt.bfloat16, name=f"w_{ncix}")
                eng.dma_start(out=t[:], in_=wT[ncix])
                wc[ncix] = t

            def load_x(g, eng):
                t = consts.tile([P, KO, GW], mybir.dt.bfloat16, name=f"x_{g}")
                eng.dma_start(out=t[:], in_=xT[g])
                xt[g] = t

            load_x(0, nc.scalar)
            load_w(0, nc.sync)
            for ncix in (1, 3, 5, 7):
                load_w(ncix, nc.scalar)
            for ncix in (2, 4, 6):
                load_w(ncix, nc.sync)
            for g in range(1, G):
                load_x(g, nc.sync if g % 2 else nc.scalar)

            osts = {}

            def evict(mi, ncix, ps):
                nc.vector.tensor_add(
                    osts[mi][:, ncix * NCHUNK : (ncix + 1) * NCHUNK],
                    ps[:],
                    bias_sb[:, ncix * NCHUNK : (ncix + 1) * NCHUNK],
                )

            def store_half(mi, half):
                eng = nc.sync if (2 * mi + half) % 2 else nc.scalar
                lo = half * (D_OUT // 2)
                eng.dma_start(
                    out=out[:, mi, lo : lo + D_OUT // 2],
                    in_=osts[mi][:, lo : lo + D_OUT // 2],
                )

            # Phase A: m-tiles 0..3, n-chunk-major (track W arrival).
            for mi in range(PHASE_A_MI):
                osts[mi] = opool.tile([P, D_OUT], mybir.dt.bfloat16, name="ost")
            for ncix in range(N_CHUNKS):
                for r in range(PHASE_A_MI):
                    ps = pspool.tile([P, NCHUNK], mybir.dt.float32, name="ps")
                    for ko in range(KO):
                        nc.tensor.matmul(
                            ps[:],
                            xt[0][:, ko, r * P : (r + 1) * P],
                            wc[ncix][:, ko, :],
                            start=(ko == 0),
                            stop=(ko == KO - 1),
                        )
                    evict(r, ncix, ps)
                if ncix == N_CHUNKS // 2 - 1:
                    for r in range(PHASE_A_MI):
                        store_half(r, 0)
            for r in range(PHASE_A_MI):
                store_half(r, 1)

            # Phase B: m-tiles 4..31, half-by-half with PSUM ping-pong.
            H = N_CHUNKS // 2
            for mi in range(PHASE_A_MI, M_TILES):
                g, r = divmod(mi, XG)
                osts[mi] = opool.tile([P, D_OUT], mybir.dt.bfloat16, name="ost")
                for half in range(2):
                    pss = [
                        pspool.tile([P, NCHUNK], mybir.dt.float32, name="ps")
                        for _ in range(H)
                    ]
                    for ko in range(KO):
                        x_st = xt[g][:, ko, r * P : (r + 1) * P]
                        for j in range(H):
                            nc.tensor.matmul(
                                pss[j][:],
                                x_st,
                                wc[half * H + j][:, ko, :],
                                start=(ko == 0),
                                stop=(ko == KO - 1),
                            )
                    for j in range(H):
                        evict(mi, half * H + j, pss[j])
                    store_half(mi, half)
    nc.finalize()

    names = (xT.name, wT.name, bias_d.name, out.name)
    _CACHE["nc"] = nc
    _CACHE["names"] = names
    return nc, names


def kernel(x: np.ndarray, weight: np.ndarray, bias: np.ndarray) -> np.ndarray:
    global LAST_RESULT
    nc, (xT_name, wT_name, bias_name, out_name) = _build()

    x = np.ascontiguousarray(x, dtype=np.float32)
    weight = np.ascontiguousarray(weight, dtype=np.float32)
    bias = np.ascontiguousarray(bias, dtype=np.float32)

    # xT[c, g, p, ko, m'] = X[c*4096 + g*512 + m', ko*128 + p]
    xT_all = np.ascontiguousarray(
        x.reshape(N_CORES, G, GW, KO, P)
        .transpose(0, 1, 4, 3, 2)
        .astype(ml_dtypes.bfloat16)
    )
    # wT[nc, p, ko, q] = W[nc*512 + q, ko*128 + p]
    wT_dev = np.ascontiguousarray(
        weight.reshape(N_CHUNKS, NCHUNK, KO, P)
        .transpose(0, 3, 2, 1)
        .astype(ml_dtypes.bfloat16)
    )

    in_maps = [
        {xT_name: xT_all[c], wT_name: wT_dev, bias_name: bias}
        for c in range(N_CORES)
    ]
    res = run_bass_kernel_spmd(nc, in_maps, list(range(N_CORES)))
    LAST_RESULT = res

    # out[p, mi, n] -> Y_shard[mi*128+p, n]; stack shards along tokens
    y = np.empty((TOK, D_OUT), dtype=np.float32)
    for c in range(N_CORES):
        o = res.results[c][out_name]  # [128, 32, 4096] bf16
        y[c * TOK_SHARD : (c + 1) * TOK_SHARD] = (
            o.astype(np.float32).transpose(1, 0, 2).reshape(TOK_SHARD, D_OUT)
        )
    return y.reshape(SEQ, BATCH, D_OUT)


# revision 8
# speedup vs baseline: 1.1185x; 1.1185x over previous
"""Column-parallel linear Y = X @ W^T + b on 8 Trainium2 NeuronCores.

Strategy: sequence-shard X across the 8 cores (4096 tokens each); every core
holds the full weight, computes its token slab against all 4096 output
features, so no collective is needed and no core re-reads another's tokens.

Device layout (per core):
  xT   [128, 8, 4096]  fp32r   xT[p, ko, m] = X_shard[m, ko*128 + p]
  wT   [128, 8, 4096]  fp32r   wT[p, ko, n] = W[n, ko*128 + p]
  bias [4096]          fp32
  out  [128, 32, 4096] fp32    out[p, mo, n] = Y_shard[mo*128 + p, n]

The PE contracts over partitions, so both operands are staged k-major.
W^T stays fully resident in SBUF (128 KB/partition); X streams through in
512-token tiles; fp32r runs the PE at 1 cycle/row (vs 4 for fp32).
"""

import numpy as np

import concourse.bass as bass
import concourse.mybir as mybir
import concourse.tile as tile
from concourse import bacc
from concourse.bass_utils import run_bass_kernel_spmd

P = 128
SEQ, BATCH, D_IN, D_OUT = 8192, 4, 1024, 4096
N_CORES = 8
TOK = SEQ * BATCH
TOK_SHARD = TOK // N_CORES     # 4096
KO = D_IN // P                 # 8
M_TILE = 512
M_OUTER = TOK_SHARD // M_TILE  # 8
M_SUB = M_TILE // P            # 4
N_TILE = 512
N_TILES = D_OUT // N_TILE      # 8

_CACHE = {}

# Last BassKernelResults, for test harnesses that want exec_time_ns.
LAST_RESULT = None


def _build():
    if "nc" in _CACHE:
        return _CACHE["nc"], _CACHE["names"]

    nc = bacc.Bacc(None, target_bir_lowering=False, debug=False)
    with tile.TileContext(nc) as tc:
        with (
            tc.tile_pool(name="dram", bufs=1, space="DRAM") as dram,
            tc.tile_pool(name="consts", bufs=1) as consts,
            tc.tile_pool(name="xpool", bufs=2) as xpool,
            tc.tile_pool(name="opool", bufs=4) as opool,
            tc.tile_pool(name="pspool", bufs=8, space="PSUM") as pspool,
        ):
            xT = dram.tile((P, KO, TOK_SHARD), mybir.dt.float32r, kind="ExternalInput")
            wT = dram.tile((P, KO, D_OUT), mybir.dt.float32r, kind="ExternalInput")
            bias_d = dram.tile((D_OUT,), mybir.dt.float32, kind="ExternalInput")
            out = dram.tile(
                (P, TOK_SHARD // P, D_OUT), mybir.dt.float32, kind="ExternalOutput"
            )

            # bias broadcast to every partition so the evict add is a plain
            # elementwise tensor_tensor
            bias_sb = consts.tile([P, D_OUT], mybir.dt.float32, name="bias_sb")
            bias_bcast = bass.AP(
                tensor=bias_d.tensor,
                offset=bias_d.offset,
                ap=[[0, P], *bias_d.ap],
            )
            nc.gpsimd.dma_start(out=bias_sb[:], in_=bias_bcast)

            def load_xm(mo):
                t = xpool.tile([P, KO, M_TILE], mybir.dt.float32r, name="xm")
                # X rides the Activation ring so xm0 and w_col0 transfer in
                # parallel on the two HWDGE rings during the lead-in
                nc.scalar.dma_start(
                    out=t[:], in_=xT[:, :, mo * M_TILE : (mo + 1) * M_TILE]
                )
                return t

            # The input DMAs drain one HW queue serially at HBM rate, so
            # emission order == arrival order. First m-tile of X goes first,
            # then the W columns in consumption order: the first matmul group
            # needs only xm0 + w_col0 (4 MB), not the whole 18.75 MB.
            xm_next = load_xm(0)
            w_cols = []
            for n in range(N_TILES):
                wc = consts.tile([P, KO, N_TILE], mybir.dt.float32r, name=f"w_{n}")
                nc.sync.dma_start(
                    out=wc[:], in_=wT[:, :, n * N_TILE : (n + 1) * N_TILE]
                )
                w_cols.append(wc)

            for mo in range(M_OUTER):
                xm = xm_next
                if mo + 1 < M_OUTER:
                    xm_next = load_xm(mo + 1)
                # n outer: consumption order matches the W column DMA arrival
                # order, so the first m-tile overlaps the weight prologue
                for n in range(N_TILES):
                    for mi in range(M_SUB):
                        ps = pspool.tile([P, N_TILE], mybir.dt.float32, name="ps")
                        for ko in range(KO):
                            nc.tensor.matmul(
                                ps[:],
                                xm[:, ko : ko + 1, mi * P : (mi + 1) * P],
                                w_cols[n][:, ko, :],
                                start=(ko == 0),
                                stop=(ko == KO - 1),
                            )
                        ot = opool.tile([P, N_TILE], mybir.dt.float32, name="ot")
                        nc.vector.tensor_add(
                            ot[:], ps[:], bias_sb[:, n * N_TILE : (n + 1) * N_TILE]
                        )
                        # outputs alternate rings by m-tile to balance the
                        # 67 MB of writes without queuing ahead of input loads
                        out_eng = nc.sync if mo % 2 else nc.scalar
                        out_eng.dma_start(
                            out=out[:, mo * M_SUB + mi, n * N_TILE : (n + 1) * N_TILE],
                            in_=ot[:],
                        )
    nc.finalize()

    names = (xT.name, wT.name, bias_d.name, out.name)
    _CACHE["nc"] = nc
    _CACHE["names"] = names
    return nc, names


def kernel(x: np.ndarray, weight: np.ndarray, bias: np.ndarray) -> np.ndarray:
    global LAST_RESULT
    nc, (xT_name, wT_name, bias_name, out_name) = _build()

    x = np.ascontiguousarray(x, dtype=np.float32)
    weight = np.ascontiguousarray(weight, dtype=np.float32)
    bias = np.ascontiguousarray(bias, dtype=np.float32)

    # [core, p, ko, m] with x[tok, k] -> xT[p, ko, m] = X_shard[m, ko*128+p]
    xT_all = np.ascontiguousarray(
        x.reshape(N_CORES, TOK_SHARD, KO, P).transpose(0, 3, 2, 1)
    )
    wT_dev = np.ascontiguousarray(weight.reshape(D_OUT, KO, P).transpose(2, 1, 0))

    in_maps = [
        {xT_name: xT_all[c], wT_name: wT_dev, bias_name: bias}
        for c in range(N_CORES)
    ]
    res = run_bass_kernel_spmd(nc, in_maps, list(range(N_CORES)))
    LAST_RESULT = res

    # out[p, mo, n] -> Y_shard[mo*128+p, n]; stack shards along tokens
    y = np.empty((TOK, D_OUT), dtype=np.float32)
    for c in range(N_CORES):
        o = res.results[c][out_name]  # [128, 32, 4096]
        y[c * TOK_SHARD : (c + 1) * TOK_SHARD] = o.transpose(1, 0, 2).reshape(
            TOK_SHARD, D_OUT
        )
    return y.reshape(SEQ, BATCH, D_OUT)



# revision 12
# speedup vs baseline: 1.3193x; 1.1795x over previous
"""Column-parallel linear Y = X @ W^T + b on 8 Trainium2 NeuronCores.

Strategy: sequence-shard X across the 8 cores (4096 tokens each); every core
holds the full weight, computes its token slab against all 4096 output
features, so no collective is needed and no core re-reads another's tokens.

v4 (mixed fp8/bf16): k-tiles 0-1 (256 of 1024 contraction rows) run as ONE
fp8-e4m3 DoubleRow matmul per output tile (the PE packs 2 fp8 weights per
cell, virtualizing the array to 256x128, streaming 2 k-rows/cycle), k-tiles
2-7 run in bf16. Scales (x*0.25, w*4) cancel in the product so both parts
accumulate into the same PSUM group. Measured norm rel err on the reference
case: 1.61e-2 (gate 2e-2). The output returns bf16; the host upcasts.

Device layout (per core):
  xT  [8, 128, 6, 512] bf16  xT[g, p, ko, m'] = X_shard[g*512+m', (ko+2)*128+p]
  wT  [8, 128, 6, 512] bf16  wT[nc, p, ko, q] = W[nc*512+q, (ko+2)*128+p]
  x8T [128, 2, 4096]   fp8e4 x8T[p, t, m] = X_shard[m, t*128+p] * 0.25
  w8T [128, 2, 4096]   fp8e4 w8T[p, t, n] = W[n, t*128+p] * 4
  bias [4096]          fp32
  out [128, 32, 4096]  bf16  out[p, mi, n] = Y_shard[mi*128+p, n]

Every DMA moves 6-8KB contiguous runs per partition (the DMA engine costs
~19ns/descriptor, so small-element transfers crawl). Inner loop: stationary
= X m-tile, moving = W [128k, 512n]; each half of the n-range accumulates
over k into 4 PSUM banks while the other half's PSUM is evicted (DVE
bias-add + bf16 cast) - ping-pong keeps the PE streaming back-to-back.
"""

import numpy as np
import ml_dtypes

import concourse.bass as bass
import concourse.mybir as mybir
import concourse.tile as tile
from concourse import bacc
from concourse.bass_utils import run_bass_kernel_spmd

P = 128
SEQ, BATCH, D_IN, D_OUT = 8192, 4, 1024, 4096
N_CORES = 8
TOK = SEQ * BATCH
TOK_SHARD = TOK // N_CORES     # 4096
KO = D_IN // P                 # 8 k-tiles total
KF8 = 2                        # k-tiles 0-1 in fp8 DoubleRow
KOB = KO - KF8                 # 6 bf16 k-tiles (real ko 2..7)
M_TILES = TOK_SHARD // P       # 32
NCHUNK = 512                   # moving-operand width (walrus ISA cap)
N_CHUNKS = D_OUT // NCHUNK     # 8
XG = 4                         # m-tiles per X DMA group
G = M_TILES // XG              # 8
X8_SCALE = 0.25                # x*0.25, w*4 -> product unscaled

_CACHE = {}

# Last BassKernelResults, for test harnesses that want exec_time_ns.
LAST_RESULT = None


def _build():
    if "nc" in _CACHE:
        return _CACHE["nc"], _CACHE["names"]

    nc = bacc.Bacc(None, target_bir_lowering=False, debug=False)
    with tile.TileContext(nc) as tc:
        with (
            tc.tile_pool(name="dram", bufs=1, space="DRAM") as dram,
            tc.tile_pool(name="consts", bufs=1) as consts,
            tc.tile_pool(name="opool", bufs=2) as opool,
            tc.tile_pool(name="pspool", bufs=8, space="PSUM") as pspool,
        ):
            xT = dram.tile((G, P, KOB, XG * P), mybir.dt.bfloat16, kind="ExternalInput")
            wT = dram.tile(
                (N_CHUNKS, P, KOB, NCHUNK), mybir.dt.bfloat16, kind="ExternalInput"
            )
            x8T = dram.tile((P, KF8, TOK_SHARD), mybir.dt.float8e4, kind="ExternalInput")
            w8T = dram.tile((P, KF8, D_OUT), mybir.dt.float8e4, kind="ExternalInput")
            bias_d = dram.tile((D_OUT,), mybir.dt.float32, kind="ExternalInput")
            out = dram.tile(
                (P, M_TILES, D_OUT), mybir.dt.bfloat16, kind="ExternalOutput"
            )

            bias_sb = consts.tile([P, D_OUT], mybir.dt.float32, name="bias_sb")
            bias_bcast = bass.AP(
                tensor=bias_d.tensor,
                offset=bias_d.offset,
                ap=[[0, P], *bias_d.ap],
            )
            nc.gpsimd.dma_start(out=bias_sb[:], in_=bias_bcast)

            wc = [None] * N_CHUNKS
            xt = [None] * G

            def load_w(ncix, eng):
                t = consts.tile([P, KOB, NCHUNK], mybir.dt.bfloat16, name=f"w_{ncix}")
                eng.dma_start(out=t[:], in_=wT[ncix])
                wc[ncix] = t

            def load_x(g, eng):
                t = consts.tile([P, KOB, XG * P], mybir.dt.bfloat16, name=f"x_{g}")
                eng.dma_start(out=t[:], in_=xT[g])
                xt[g] = t

            x8 = consts.tile([P, KF8, TOK_SHARD], mybir.dt.float8e4, name="x8")
            w8 = consts.tile([P, KF8, D_OUT], mybir.dt.float8e4, name="w8")

            load_x(0, nc.scalar)
            load_w(0, nc.sync)
            nc.scalar.dma_start(out=x8[:], in_=x8T[:])
            nc.sync.dma_start(out=w8[:], in_=w8T[:])
            for ncix in (1, 3, 5, 7):
                load_w(ncix, nc.scalar)
            for ncix in (2, 4, 6):
                load_w(ncix, nc.sync)
            for g in range(1, G):
                load_x(g, nc.sync if g % 2 else nc.scalar)

            H = N_CHUNKS // 2
            for mi in range(M_TILES):
                g, r = divmod(mi, XG)
                ost = opool.tile([P, D_OUT], mybir.dt.bfloat16, name="ost")
                for half in range(2):
                    pss = [
                        pspool.tile([P, NCHUNK], mybir.dt.float32, name="ps")
                        for _ in range(H)
                    ]
                    # fp8 DoubleRow: one K=256 matmul opens each group
                    x8_st = x8[:, :, mi * P : (mi + 1) * P]
                    for j in range(H):
                        ncix = half * H + j
                        nc.tensor.matmul(
                            pss[j][:],
                            x8_st,
                            w8[:, :, ncix * NCHUNK : (ncix + 1) * NCHUNK],
                            start=True,
                            stop=False,
                            perf_mode=mybir.MatmulPerfMode.DoubleRow,
                        )
                    for ko in range(KOB):
                        x_st = xt[g][:, ko, r * P : (r + 1) * P]
                        for j in range(H):
                            ncix = half * H + j
                            nc.tensor.matmul(
                                pss[j][:],
                                x_st,
                                wc[ncix][:, ko, :],
                                start=False,
                                stop=(ko == KOB - 1),
                            )
                    for j in range(H):
                        ncix = half * H + j
                        nc.vector.tensor_add(
                            ost[:, ncix * NCHUNK : (ncix + 1) * NCHUNK],
                            pss[j][:],
                            bias_sb[:, ncix * NCHUNK : (ncix + 1) * NCHUNK],
                        )
                out_eng = nc.sync if mi % 2 else nc.scalar
                out_eng.dma_start(out=out[:, mi, :], in_=ost[:])
    nc.finalize()

    names = (xT.name, wT.name, x8T.name, w8T.name, bias_d.name, out.name)
    _CACHE["nc"] = nc
    _CACHE["names"] = names
    return nc, names


def kernel(x: np.ndarray, weight: np.ndarray, bias: np.ndarray) -> np.ndarray:
    global LAST_RESULT
    nc, (xT_name, wT_name, x8_name, w8_name, bias_name, out_name) = _build()

    x = np.ascontiguousarray(x, dtype=np.float32)
    weight = np.ascontiguousarray(weight, dtype=np.float32)
    bias = np.ascontiguousarray(bias, dtype=np.float32)

    xr = x.reshape(N_CORES, G, XG * P, KO, P)
    # bf16 part: real ko 2..7
    xT_all = np.ascontiguousarray(
        xr[:, :, :, KF8:, :].transpose(0, 1, 4, 3, 2).astype(ml_dtypes.bfloat16)
    )
    # fp8 part: ko 0..1, scaled by 1/4; [c, p, t, m]
    x8_all = np.ascontiguousarray(
        (x.reshape(N_CORES, TOK_SHARD, KO, P)[:, :, :KF8, :] * X8_SCALE)
        .transpose(0, 3, 2, 1)
        .astype(ml_dtypes.float8_e4m3)
    )

    wr = weight.reshape(N_CHUNKS, NCHUNK, KO, P)
    wT_dev = np.ascontiguousarray(
        wr[:, :, KF8:, :].transpose(0, 3, 2, 1).astype(ml_dtypes.bfloat16)
    )
    w8_dev = np.ascontiguousarray(
        (weight.reshape(D_OUT, KO, P)[:, :KF8, :] / X8_SCALE)
        .transpose(2, 1, 0)
        .astype(ml_dtypes.float8_e4m3)
    )

    in_maps = [
        {
            xT_name: xT_all[c],
            wT_name: wT_dev,
            x8_name: x8_all[c],
            w8_name: w8_dev,
            bias_name: bias,
        }
        for c in range(N_CORES)
    ]
    res = run_bass_kernel_spmd(nc, in_maps, list(range(N_CORES)))
    LAST_RESULT = res

    # out[p, mi, n] -> Y_shard[mi*128+p, n]; stack shards along tokens
    y = np.empty((TOK, D_OUT), dtype=np.float32)
    for c in range(N_CORES):
        o = res.results[c][out_name]  # [128, 32, 4096] bf16
        y[c * TOK_SHARD : (c + 1) * TOK_SHARD] = (
            o.astype(np.float32).transpose(1, 0, 2).reshape(TOK_SHARD, D_OUT)
        )
    return y.reshape(SEQ, BATCH, D_OUT)
